# revision 1
# baseline (speedup 1.0000x reference)
"""Bilateral filter (7x7, dilation 1) Trainium2 Bass kernel.

Problem: input [2, 18, 1024, 1024] f32.
  filterable = input[:, :8]; params = -(input[:, 8:]**2)
  range coeffs = params[:, :8], sx = params[:, 8], sy = params[:, 9]
  out[c] = sum_taps w * f_c(shifted) / sum_taps w, c < 3
  w = exp(sum_c r_c (fn_c - f_c)^2 + sx dx^2 + sy dy^2), OOB taps masked.

Sharding: data-parallel over (batch, H): 8 cores, each gets 256 rows of one
batch image (+3 halo rows each side, sentinel-padded host-side).  Out-of-image
taps get weight exactly 0 because the sentinel (1e18) drives the quadratic
form to -huge and exp underflows to +0.

Per-core layout: H rows on partitions (128 x 2 blocks), W in chunks of 256 on
the free axis with the 8 filterable channels interleaved (x*8+c).  Row shifts
(oy) come from 7 row-shifted tile copies; column shifts (ox) are free-axis
offsets into the 6-column halo.

Engine split per tap: DVE sub/reduce/adds, ACT square/exp, GPSIMD r*d^2.
"""

import sys

if "/opt/trn_rl_repo" not in sys.path:
    sys.path.insert(0, "/opt/trn_rl_repo")

import numpy as np

import concourse.bass as bass
import concourse.mybir as mybir
from concourse.bacc import Bacc
from concourse.tile import TileContext

FP32 = mybir.dt.float32

B, C_ALL, H, W = 2, 18, 1024, 1024
CF = 8                      # filterable channels
CO = 3                      # output channels
KS, RAD = 7, 3
HC = H * B // 8             # 256 output rows per core
HIN = HC + 2 * RAD          # 262 input rows per core (halo padded host-side)
WC = 256                    # W chunk
NW = W // WC                # 4
NHB = HC // 128             # 2
SENT = 1.0e18               # sentinel padding value -> tap weight exp(-huge)=0
D2 = [9.0, 4.0, 1.0, 0.0, 1.0, 4.0, 9.0]   # (k-3)^2 for k in 0..6
D2IDX = [3, 2, 1, 0, 1, 2, 3]              # index into [0,1,4,9]
D2VALS = [0.0, 1.0, 4.0, 9.0]

_CACHED = {}
TAP_SET = None   # optional [(i,j)] subset for debugging


def _ilv(ap, n, c=CF):
    """View flat [128, n*c] region as [128, n, c] (channel-interleaved)."""
    return ap.rearrange("p (x c) -> p x c", c=c)


def build_nc(macros=None):
    nc = Bacc()
    x = nc.dram_tensor("x", [C_ALL, HIN, W], FP32, kind="ExternalInput")
    y = nc.dram_tensor("y", [CO, HC, W], FP32, kind="ExternalOutput")

    if macros is None:
        macros = [(hb, wck) for hb in range(NHB) for wck in range(NW)]
    with TileContext(nc) as tc:
        with (
            tc.tile_pool(name="fpool", bufs=1) as fpool,
            tc.tile_pool(name="cpool", bufs=1) as cpool,
            tc.tile_pool(name="dpool", bufs=5) as dpool,
            tc.tile_pool(name="spool", bufs=4) as spool,
            tc.tile_pool(name="ppool", bufs=1, space="PSUM") as ppool,
        ):
            for hb, wcki in macros:
                _macro(nc, tc, x, y, fpool, cpool, dpool, spool, ppool, hb, wcki)
    nc.compile()
    return nc


def _macro(nc, tc, x, y, fpool, cpool, dpool, spool, ppool, hb, wck):
    w0 = wck * WC
    r0 = hb * 128
    wtile = WC + 2 * RAD
    # tile col t  <->  image col w0 - 3 + t
    lo = RAD if wck == 0 else 0
    hi = wtile - RAD if wck == NW - 1 else wtile

    # ---- load + interleave the 7 row-shifted filterable tile sets ----
    F = []
    for oy in range(KS):
        Fi = fpool.tile([128, wtile * CF], FP32, tag=f"F{oy}", bufs=1,
                        name=f"F{oy}_{hb}_{wck}")
        for c in range(CF):
            pl = fpool.tile([128, wtile], FP32, tag="pl", bufs=3,
                            name=f"pl_{hb}_{wck}_{oy}_{c}")
            if lo > 0:
                nc.gpsimd.memset(pl[:, 0:lo], SENT)
            if hi < wtile:
                nc.gpsimd.memset(pl[:, hi:wtile], SENT)
            nc.sync.dma_start(
                out=pl[:, lo:hi],
                in_=x[c, r0 + oy : r0 + oy + 128, w0 - RAD + lo : w0 - RAD + hi],
            )
            # interleave: Fi[p, t*8+c] = pl[p, t]   (ACT, strided out)
            nc.scalar.copy(_ilv(Fi[:], wtile)[:, :, c], pl[:])
        F.append(Fi)
    Fc = _ilv(F[RAD][:, RAD * CF : (RAD + WC) * CF], WC)      # center view

    # ---- params: R (interleaved), sx2, sy2 ----
    R = cpool.tile([128, WC * CF], FP32, tag="R", name=f"R_{hb}_{wck}")
    for c in range(CF):
        pp = fpool.tile([128, WC], FP32, tag="pp", bufs=2,
                        name=f"pp_{hb}_{wck}_{c}")
        nc.sync.dma_start(
            out=pp[:], in_=x[CF + c, r0 + RAD : r0 + RAD + 128, w0 : w0 + WC])
        nc.vector.scalar_tensor_tensor(
            _ilv(R[:], WC)[:, :, c], pp[:], -1.0, pp[:],
            mybir.AluOpType.mult, mybir.AluOpType.mult)
    sxy2 = cpool.tile([128, 2 * WC], FP32, tag="sxy2", name=f"sxy2_{hb}_{wck}")
    for k in range(2):
        pp = fpool.tile([128, WC], FP32, tag="pp", bufs=2,
                        name=f"pps_{hb}_{wck}_{k}")
        nc.sync.dma_start(
            out=pp[:], in_=x[2 * CF + k, r0 + RAD : r0 + RAD + 128, w0 : w0 + WC])
        nc.vector.scalar_tensor_tensor(
            sxy2[:, k * WC : (k + 1) * WC], pp[:], -1.0, pp[:],
            mybir.AluOpType.mult, mybir.AluOpType.mult)
    sx2 = sxy2[:, 0:WC]
    sy2 = sxy2[:, WC : 2 * WC]

    # ---- Asp[a][b] = a*sx2 + b*sy2  (spatial log-weight, 16 combos) ----
    Ab = spool.tile([128, 4 * WC], FP32, tag="Ab", bufs=2, name=f"Ab_{hb}_{wck}")
    for bi, bval in enumerate(D2VALS):
        nc.vector.tensor_scalar_mul(
            Ab[:, bi * WC : (bi + 1) * WC], sy2, float(bval))
    Asp = cpool.tile([128, 16 * WC], FP32, tag="Asp", name=f"Asp_{hb}_{wck}")
    for ai, aval in enumerate(D2VALS):
        for bi in range(4):
            nc.vector.scalar_tensor_tensor(
                Asp[:, (ai * 4 + bi) * WC : (ai * 4 + bi + 1) * WC],
                sx2, float(aval), Ab[:, bi * WC : (bi + 1) * WC],
                mybir.AluOpType.mult, mybir.AluOpType.add)

    # ---- accumulators ----
    acc = cpool.tile([128, WC * CO], FP32, tag="acc", name=f"acc_{hb}_{wck}")
    wsum = cpool.tile([128, WC], FP32, tag="wsum", name=f"wsum_{hb}_{wck}")
    nc.gpsimd.memset(acc[:], 0.0)
    nc.gpsimd.memset(wsum[:], 0.0)

    # ---- 49 taps ----
    taps = TAP_SET if TAP_SET is not None else [(i, j) for i in range(KS) for j in range(KS)]
    for i, j in taps:            # oy = i - 3, ox = j - 3
        if True:
            Fi = F[i]
            sh = _ilv(Fi[:, j * CF : (j + WC) * CF], WC)     # shifted read
            d = dpool.tile([128, WC * CF], FP32, tag="d",
                           name=f"d_{hb}_{wck}_{i}_{j}")
            nc.vector.tensor_sub(_ilv(d[:], WC), sh, Fc)
            nc.scalar.activation(d[:], d[:], mybir.ActivationFunctionType.Square)
            nc.gpsimd.tensor_mul(d[:], R[:], d[:])
            s = spool.tile([128, WC], FP32, tag="s",
                           name=f"s_{hb}_{wck}_{i}_{j}")
            nc.vector.tensor_reduce(s[:], _ilv(d[:], WC),
                                    axis=mybir.AxisListType.X,
                                    op=mybir.AluOpType.add)
            k = (D2IDX[j] * 4 + D2IDX[i]) * WC
            nc.vector.tensor_add(s[:], s[:], Asp[:, k : k + WC])
            w_t = spool.tile([128, WC], FP32, tag="w",
                             name=f"w_{hb}_{wck}_{i}_{j}")
            nc.scalar.activation(w_t[:], s[:], mybir.ActivationFunctionType.Exp)
            nc.vector.tensor_add(wsum[:], wsum[:], w_t[:])
            t3 = spool.tile([128, WC * CO], FP32, tag="t3",
                            name=f"t3_{hb}_{wck}_{i}_{j}")
            w_b = w_t[:].unsqueeze(2).broadcast_to([128, WC, CO])
            f3 = _ilv(Fi[:, j * CF : (j + WC) * CF], WC)[:, :, 0:CO]
            nc.vector.tensor_mul(_ilv(t3[:], WC, CO), w_b, f3)
            nc.vector.tensor_add(acc[:], acc[:], t3[:])

    # ---- out = acc / wsum ----
    rec = spool.tile([128, WC], FP32, tag="s", name=f"rec_{hb}_{wck}")
    nc.vector.reciprocal(rec[:], wsum[:])
    out3 = spool.tile([128, WC * CO], FP32, tag="t3", name=f"out3_{hb}_{wck}")
    rec_b = rec[:].unsqueeze(2).broadcast_to([128, WC, CO])
    nc.vector.tensor_mul(_ilv(out3[:], WC, CO), rec_b, _ilv(acc[:], WC, CO))
    for c in range(CO):
        oc = spool.tile([128, WC], FP32, tag="oc", name=f"oc_{hb}_{wck}_{c}")
        nc.scalar.copy(oc[:], _ilv(out3[:], WC, CO)[:, :, c])
        nc.sync.dma_start(out=y[c, r0 : r0 + 128, w0 : w0 + WC], in_=oc[:])


def shard_inputs(input):
    """input [2,18,1024,1024] -> 8 per-core slabs [18, 262, 1024]."""
    input = np.asarray(input, dtype=np.float32)
    per_b = 4
    rows = H // per_b
    in_maps = []
    for core in range(8):
        b, q = divmod(core, per_b)
        r0 = q * rows
        slab = np.full((C_ALL, HIN, W), SENT, dtype=np.float32)
        s_lo = max(r0 - RAD, 0)
        s_hi = min(r0 + rows + RAD, H)
        slab[:, s_lo - (r0 - RAD) : s_hi - (r0 - RAD), :] = input[b, :, s_lo:s_hi, :]
        in_maps.append({"x": np.ascontiguousarray(slab)})
    return in_maps


def assemble(results):
    out = np.empty((B, CO, H, W), dtype=np.float32)
    rows = H // 4
    for core in range(8):
        b, q = divmod(core, 4)
        out[b, :, q * rows : (q + 1) * rows, :] = results[core]["y"]
    return out


def kernel(input):
    from concourse.bass_utils import run_bass_kernel_spmd

    if "nc" not in _CACHED:
        _CACHED["nc"] = build_nc()
    in_maps = shard_inputs(input)
    res = run_bass_kernel_spmd(_CACHED["nc"], in_maps, list(range(8)))
    return assemble(res.results)



# revision 4
# speedup vs baseline: 1.7822x; 1.7822x over previous
"""Bilateral filter (7x7, dilation 1) Trainium2 Bass kernel — v2.

Problem: input [2, 18, 1024, 1024] f32.
  filterable = input[:, :8]; params = -(input[:, 8:]**2)
  range coeffs = params[:, :8], sx = params[:, 8], sy = params[:, 9]
  out[c] = sum_taps w * f_c(shifted) / sum_taps w, c < 3
  w = exp(sum_c r_c (fn_c - f_c)^2 + sx dx^2 + sy dy^2), OOB taps masked.

Sharding: data-parallel over (batch, H): 8 cores, each 256 rows of one batch
image (+3 halo rows AND +3 halo cols, sentinel-padded host-side).

v2 design vs v1:
  * fp16 heavy path (DVE tensor_tensor 2x mode; validated rel err 2.3e-4).
  * channel-PLANAR tiles [128 rows, 8 ch, W+6 cols] — no interleave copies;
    every tree-reduce level is a stride-1 fp16 TT add (all 2x).
  * finite sentinel 240.0: d^2 = 57600 stays finite in fp16, r*d^2
    overflows to -inf only via genuinely negative products, exp -> +0.
    No 0*inf NaN path.
  * engine split per tap: DVE sub/rmul/tree/+Asp, ACT square/exp,
    GPSIMD wsum/numerator accumulation.
  * center tap folded into init (w=1: wsum=1, acc=f3).
"""

import sys

if "/opt/trn_rl_repo" not in sys.path:
    sys.path.insert(0, "/opt/trn_rl_repo")

import numpy as np

import concourse.bass as bass
import concourse.mybir as mybir
from concourse.bacc import Bacc
from concourse.tile import TileContext

FP32 = mybir.dt.float32
FP16 = mybir.dt.float16

B, C_ALL, H, W = 2, 18, 1024, 1024
CF = 8                      # filterable channels
CO = 3                      # output channels
KS, RAD = 7, 3
HC = H * B // 8             # 256 output rows per core
HIN = HC + 2 * RAD          # 262 input rows per core (halo padded host-side)
WIN = W + 2 * RAD           # 1030 input cols per core (halo padded host-side)
WC = 512                    # W chunk
NW = W // WC                # 2
NHB = HC // 128             # 2
WT = WC + 2 * RAD           # 518 = chunk + col halo
SENT = 240.0                # sentinel: 240^2 = 57600 finite in fp16
D2IDX = {0: 3, 1: 2, 2: 1, 3: 0, 4: 1, 5: 2, 6: 3}   # |k-3| -> index trick
D2VALS = [0.0, 1.0, 4.0, 9.0]
IDX4 = [3, 2, 1, 0, 1, 2, 3]                          # (k-3)^2 class index

_CACHED = {}
TAP_SET = None   # optional [(i,j)] subset for debugging


def build_nc(macros=None):
    nc = Bacc()
    x = nc.dram_tensor("x", [C_ALL, HIN, WIN], FP32, kind="ExternalInput")
    y = nc.dram_tensor("y", [CO, HC, W], FP32, kind="ExternalOutput")

    if macros is None:
        macros = [(hb, wck) for hb in range(NHB) for wck in range(NW)]
    with TileContext(nc) as tc:
        with (
            tc.tile_pool(name="fpool", bufs=1) as fpool,
            tc.tile_pool(name="stpool", bufs=2) as stpool,
            tc.tile_pool(name="cpool", bufs=1) as cpool,
            tc.tile_pool(name="dpool", bufs=2) as dpool,
            tc.tile_pool(name="spool", bufs=3) as spool,
        ):
            for hb, wcki in macros:
                _macro(nc, tc, x, y, fpool, stpool, cpool, dpool, spool,
                       hb, wcki)
    nc.compile()
    return nc


def _macro(nc, tc, x, y, fpool, stpool, cpool, dpool, spool, hb, wck):
    w0 = wck * WC
    r0 = hb * 128

    # ---- load + convert the 7 row-shifted planar F tile sets (fp16) ----
    F = []       # F[oy]: [128, CF, WT] fp16, rows r0+oy .. r0+oy+127 (slab)
    for oy in range(KS):
        st = stpool.tile([128, CF * WT], FP32, tag="stage", bufs=1,
                         name=f"st_{hb}_{wck}_{oy}")
        st3 = st[:].rearrange("p (c x) -> p c x", x=WT)
        for c in range(CF):
            nc.sync.dma_start(
                out=st3[:, c, :],
                in_=x[c, r0 + oy : r0 + oy + 128, w0 : w0 + WT],
            )
        bufs = 2 if oy == RAD else 1
        Fi = fpool.tile([128, CF * WT], FP16, tag=f"F{oy}", bufs=bufs,
                        name=f"F{oy}_{hb}_{wck}")
        nc.vector.tensor_copy(Fi[:], st[:])
        F.append(Fi)

    def f3d(oy):
        return F[oy][:].rearrange("p (c x) -> p c x", x=WT)

    Fc = f3d(RAD)[:, :, RAD : RAD + WC]          # center view [128, 8, WC]

    # ---- params: R = -(p*p) fp16 planar, sx2, sy2, Asp ----
    pst = stpool.tile([128, CF * WC], FP32, tag="pstage", bufs=1,
                      name=f"pst_{hb}_{wck}")
    pst3 = pst[:].rearrange("p (c x) -> p c x", x=WC)
    for c in range(CF):
        nc.sync.dma_start(
            out=pst3[:, c, :],
            in_=x[CF + c, r0 + RAD : r0 + RAD + 128, w0 + RAD : w0 + RAD + WC])
    R = cpool.tile([128, CF * WC], FP16, tag="R", name=f"R_{hb}_{wck}")
    nc.vector.scalar_tensor_tensor(
        R[:], pst[:], -1.0, pst[:], mybir.AluOpType.mult, mybir.AluOpType.mult)

    sst = stpool.tile([128, 2 * WC], FP32, tag="sstage", bufs=1,
                      name=f"sst_{hb}_{wck}")
    for k in range(2):
        nc.sync.dma_start(
            out=sst[:, k * WC : (k + 1) * WC],
            in_=x[2 * CF + k, r0 + RAD : r0 + RAD + 128,
                  w0 + RAD : w0 + RAD + WC])
    sxy = cpool.tile([128, 2 * WC], FP16, tag="sxy", name=f"sxy_{hb}_{wck}")
    nc.vector.scalar_tensor_tensor(
        sxy[:], sst[:], -1.0, sst[:], mybir.AluOpType.mult,
        mybir.AluOpType.mult)
    sx2 = sxy[:, 0:WC]
    sy2 = sxy[:, WC : 2 * WC]

    # Asp[ai*4+bi] = A*sx2 + B*sy2 for A,B in {0,1,4,9}
    syb = spool.tile([128, 4 * WC], FP16, tag="syb", bufs=1,
                     name=f"syb_{hb}_{wck}")
    for bi, bval in enumerate(D2VALS):
        nc.vector.tensor_scalar_mul(
            syb[:, bi * WC : (bi + 1) * WC], sy2, float(bval))
    Asp = cpool.tile([128, 16 * WC], FP16, tag="Asp", name=f"Asp_{hb}_{wck}")
    for ai, aval in enumerate(D2VALS):
        for bi in range(4):
            nc.vector.scalar_tensor_tensor(
                Asp[:, (ai * 4 + bi) * WC : (ai * 4 + bi + 1) * WC],
                sx2, float(aval), syb[:, bi * WC : (bi + 1) * WC],
                mybir.AluOpType.mult, mybir.AluOpType.add)

    # ---- accumulators: center tap folded in (w=1) ----
    acc = cpool.tile([128, CO * WC], FP32, tag="acc", name=f"acc_{hb}_{wck}")
    wsum = cpool.tile([128, WC], FP32, tag="wsum", name=f"wsum_{hb}_{wck}")
    nc.vector.tensor_copy(
        acc[:].rearrange("p (c x) -> p c x", x=WC), Fc[:, 0:CO, :])
    nc.gpsimd.memset(wsum[:], 1.0)

    # ---- 48 off-center taps ----
    taps = TAP_SET if TAP_SET is not None else [
        (i, j) for i in range(KS) for j in range(KS) if (i, j) != (RAD, RAD)]
    for i, j in taps:            # oy = i - 3, ox = j - 3
        sh = f3d(i)[:, :, j : j + WC]                 # shifted view
        d = dpool.tile([128, CF * WC], FP16, tag="d",
                       name=f"d_{hb}_{wck}_{i}_{j}")
        nc.vector.tensor_sub(
            d[:].rearrange("p (c x) -> p c x", x=WC), sh, Fc)
        d2 = dpool.tile([128, CF * WC], FP16, tag="d2",
                        name=f"d2_{hb}_{wck}_{i}_{j}")
        nc.scalar.activation(d2[:], d[:], mybir.ActivationFunctionType.Square)
        rd = d
        nc.vector.tensor_mul(rd[:], R[:], d2[:])
        rd3 = rd[:].rearrange("p (c x) -> p c x", x=WC)
        t1 = spool.tile([128, 4 * WC], FP16, tag="t1", bufs=2,
                        name=f"t1_{hb}_{wck}_{i}_{j}")
        nc.vector.tensor_add(
            t1[:].rearrange("p (c x) -> p c x", x=WC),
            rd3[:, 0:4, :], rd3[:, 4:8, :])
        t13 = t1[:].rearrange("p (c x) -> p c x", x=WC)
        t2 = spool.tile([128, 2 * WC], FP16, tag="t2", bufs=2,
                        name=f"t2_{hb}_{wck}_{i}_{j}")
        nc.vector.tensor_add(
            t2[:].rearrange("p (c x) -> p c x", x=WC),
            t13[:, 0:2, :], t13[:, 2:4, :])
        st_ = spool.tile([128, WC], FP16, tag="s",
                         name=f"s_{hb}_{wck}_{i}_{j}")
        k16 = (IDX4[j] * 4 + IDX4[i]) * WC
        nc.vector.tensor_add(st_[:], t2[:, 0:WC], t2[:, WC : 2 * WC])
        stt = spool.tile([128, WC], FP16, tag="s",
                         name=f"stt_{hb}_{wck}_{i}_{j}")
        nc.vector.tensor_add(stt[:], st_[:], Asp[:, k16 : k16 + WC])
        w_t = spool.tile([128, WC], FP16, tag="w",
                         name=f"w_{hb}_{wck}_{i}_{j}")
        nc.scalar.activation(w_t[:], stt[:], mybir.ActivationFunctionType.Exp)
        # accumulate on GPSIMD
        nc.gpsimd.tensor_add(wsum[:], wsum[:], w_t[:])
        t3 = spool.tile([128, CO * WC], FP16, tag="t3", bufs=2,
                        name=f"t3_{hb}_{wck}_{i}_{j}")
        w_b = w_t[:].unsqueeze(1).broadcast_to([128, CO, WC])
        nc.gpsimd.tensor_mul(
            t3[:].rearrange("p (c x) -> p c x", x=WC), w_b,
            f3d(i)[:, 0:CO, j : j + WC])
        nc.gpsimd.tensor_add(acc[:], acc[:], t3[:])

    # ---- out = acc / wsum ----
    rec = spool.tile([128, WC], FP32, tag="rec", bufs=1,
                     name=f"rec_{hb}_{wck}")
    nc.vector.reciprocal(rec[:], wsum[:])
    out3 = spool.tile([128, CO * WC], FP32, tag="out3", bufs=1,
                      name=f"out3_{hb}_{wck}")
    rec_b = rec[:].unsqueeze(1).broadcast_to([128, CO, WC])
    nc.vector.tensor_mul(
        out3[:].rearrange("p (c x) -> p c x", x=WC), rec_b,
        acc[:].rearrange("p (c x) -> p c x", x=WC))
    o3 = out3[:].rearrange("p (c x) -> p c x", x=WC)
    for c in range(CO):
        nc.sync.dma_start(out=y[c, r0 : r0 + 128, w0 : w0 + WC],
                          in_=o3[:, c, :])


def shard_inputs(input):
    """input [2,18,1024,1024] -> 8 per-core slabs [18, 262, 1030]."""
    input = np.asarray(input, dtype=np.float32)
    per_b = 4
    rows = H // per_b
    in_maps = []
    for core in range(8):
        b, q = divmod(core, per_b)
        r0 = q * rows
        slab = np.full((C_ALL, HIN, WIN), SENT, dtype=np.float32)
        s_lo = max(r0 - RAD, 0)
        s_hi = min(r0 + rows + RAD, H)
        slab[:, s_lo - (r0 - RAD) : s_hi - (r0 - RAD), RAD : RAD + W] = \
            input[b, :, s_lo:s_hi, :]
        in_maps.append({"x": np.ascontiguousarray(slab)})
    return in_maps


def assemble(results):
    out = np.empty((B, CO, H, W), dtype=np.float32)
    rows = H // 4
    for core in range(8):
        b, q = divmod(core, 4)
        out[b, :, q * rows : (q + 1) * rows, :] = results[core]["y"]
    return out


def kernel(input):
    from concourse.bass_utils import run_bass_kernel_spmd

    if "nc" not in _CACHED:
        _CACHED["nc"] = build_nc()
    in_maps = shard_inputs(input)
    res = run_bass_kernel_spmd(_CACHED["nc"], in_maps, list(range(8)))
    return assemble(res.results)
